# revision 3
# baseline (speedup 1.0000x reference)
"""Self-contained DNC kernel entry point: kernel(**inputs) -> y [16, 32, 256].

Shards batch 16 across 8 NeuronCores (2 per core), runs the Bass/Tile DNC
kernel, gathers outputs. See dnc_kernel-equivalent body inlined below.
"""
import sys
sys.path.insert(0, "/root/problem")
import numpy as np

import dnc_kernel as DK
from concourse.bass_utils import run_bass_kernel_spmd

LAST_EXEC_NS = None
_CACHED = {}


def kernel(**inputs):
    global LAST_EXEC_NS
    inputs = {k: np.asarray(v) for k, v in inputs.items()}
    Btot, T, INd = inputs["x"].shape
    n_cores = 8
    Bper = Btot // n_cores

    in_maps = []
    for c in range(n_cores):
        hp, _, _ = DK.host_prep(inputs, c * Bper, (c + 1) * Bper)
        in_maps.append(hp)

    key = (T,)
    if key not in _CACHED:
        _CACHED[key] = DK.build(T, n_devices=n_cores, gpsimd_stt=False,
                                dump_states=False)
    nc = _CACHED[key]

    import os
    trace = bool(os.environ.get("DNC_TRACE"))
    res = run_bass_kernel_spmd(nc, in_maps, core_ids=list(range(n_cores)),
                               trace=trace)
    LAST_EXEC_NS = res.exec_time_ns
    y = np.concatenate([r["y"] for r in res.results], axis=0)  # [16, T, 256]
    return y.astype(np.float32)
